# revision 14
# baseline (speedup 1.0000x reference)
"""Criss-cross attention (nn_CC_attention) Trainium2 kernel.

Sharding: pure data parallel over batch B=8 across 8 NeuronCores; the only
cross-core coupling is the global min/max of energy, exchanged via a tiny
AllReduce(max) of (max, -min).

Host-side staging (layout/precision only; all model compute is on-device):
  t1b = bf16(tensor1) as (H, C, W)   -- keys source
  t2h = fp16(tensor2) as (H, C, W)   -- carries the exact +tensor2 path
  t2t = bf16(tensor2) as (W, C, H)   -- pre-transposed copy for the W branch
  out is produced as fp16 (H, C, W), host transposes back to (C, H, W) fp32.

Per-core device algorithm:
  phase 1 (stream 32 groups of 8 channels):
    kW[c][h,k] = avg-pool_w(t1)  (DVE reduce + scale->fp16)
    kH[c][w,k] = t1b[c].T @ P    (PE, t1 as stationary; P = pooling matrix /8)
    eW[w,k] += t2h[c].T @ kW[c]  (PE, t2h stationary, K=h)
    eH[h,k] += t2t[c].T @ kH[c]  (PE, t2t stationary, K=w)
  boundary:
    local (max,-min) -> AllReduce(max) -> global range; exp on ACT; sums;
    att -> A_HT[h',h] = 0.0625*att_H[h,h'//8] + I (fp16)
           M_W[w',w]  = 0.0625*att_W[w,w'//8]     (bf16)
    (0.5 gamma is folded into the 0.0625; the full +tensor2 rides A_HT's I)
  phase 2 (per group):
    psum[h,(c,w)] = A_HT.T @ t2h[g]  (N=512 x2)
                  += t2t[c].T @ M_W  (per c)
    = 0.5*out_H + 0.5*out_W + tensor2 ;  ACT copy -> fp16 -> DMA out.
PE warm-up bursts (dummy matmuls) run at kernel start and during the
collective so the HAM clock gate is open (2.4 GHz) for both phases.
"""

import numpy as np
from contextlib import ExitStack

import ml_dtypes
import concourse.bass as bass
import concourse.tile as tile
from concourse import bacc, bass_isa, mybir

B, C, H, W, POOL = 8, 256, 128, 128, 8
KH, KW = H // POOL, W // POOL  # 16, 16
NCORES = 8
G = 8  # channels per group tile
NWARM = 56  # dummy matmuls per PE warm-up burst

F32 = mybir.dt.float32
F16 = mybir.dt.float16
BF16 = mybir.dt.bfloat16
F8 = mybir.dt.float8e4
BF_NP = ml_dtypes.bfloat16
F8_NP = ml_dtypes.float8_e4m3


def host_constants():
    pool_m = np.zeros((H, KH), np.float32)
    for k in range(KH):
        pool_m[k * POOL:(k + 1) * POOL, k] = 1.0 / POOL
    expmat = np.zeros((KH, H), np.float32)
    for k in range(KH):
        expmat[k, k * POOL:(k + 1) * POOL] = 0.5 / POOL  # 0.0625
    return {
        "pool16": pool_m.astype(F8_NP),
        "ident16": np.eye(H, dtype=np.float32).astype(BF_NP),
        "expmat": expmat.astype(BF_NP),
        "eyefull": np.eye(H, dtype=np.float32),
    }


def build(c_total=C, ncores=NCORES):
    assert c_total % G == 0
    ngroups = c_total // G
    nc = bacc.Bacc(trn_type="TRN2", target_bir_lowering=False, debug=False,
                   num_devices=ncores)

    t1b = nc.dram_tensor("t1b", [H, c_total, W], F8, kind="ExternalInput").ap()
    t2h = nc.dram_tensor("t2h", [H, c_total, W], F16, kind="ExternalInput").ap()
    t2t = nc.dram_tensor("t2t", [W, c_total, H], BF16, kind="ExternalInput").ap()
    pool16 = nc.dram_tensor("pool16", [H, KH], F8, kind="ExternalInput").ap()
    ident16 = nc.dram_tensor("ident16", [H, W], BF16, kind="ExternalInput").ap()
    expmat = nc.dram_tensor("expmat", [KH, H], BF16, kind="ExternalInput").ap()
    eyefull = nc.dram_tensor("eyefull", [H, W], F32, kind="ExternalInput").ap()
    out = nc.dram_tensor("out", [H, c_total, W], F16, kind="ExternalOutput").ap()

    with tile.TileContext(nc) as tc, ExitStack() as top:
        # ---- constants ----
        cpool = top.enter_context(tc.tile_pool(name="consts", bufs=1))
        c_pool16 = cpool.tile([H, KH], F8, tag="pool16")
        nc.sync.dma_start(c_pool16[:], pool16[:])
        c_ident = cpool.tile([H, W], BF16, tag="ident16")
        nc.sync.dma_start(c_ident[:], ident16[:])
        c_expmat = cpool.tile([KH, H], BF16, tag="expmat")
        nc.sync.dma_start(c_expmat[:], expmat[:])
        c_eye = cpool.tile([H, W], F32, tag="eyefull")
        nc.sync.dma_start(c_eye[:], eyefull[:])

        resq = top.enter_context(tc.tile_pool(name="resq", bufs=ngroups))
        resqT = top.enter_context(tc.tile_pool(name="resqT", bufs=ngroups))
        t2q_tiles, t2t_tiles = [], []

        psb = ExitStack()  # psum pools released before phase 2
        ps_e = psb.enter_context(tc.tile_pool(name="ps_e", bufs=1, space="PSUM"))
        ps_eW = ps_e.tile([W, KW], F32, tag="eW")
        ps_eH = ps_e.tile([H, KH], F32, tag="eH")
        ps_warm = psb.enter_context(tc.tile_pool(name="ps_warm", bufs=1, space="PSUM"))
        ps_w = ps_warm.tile([H, W], F32, tag="warm")

        spool = top.enter_context(tc.tile_pool(name="soft", bufs=1))
        dram = top.enter_context(tc.tile_pool(name="dram", bufs=1, space="DRAM"))

        # PE warm-up burst #1 (no data deps -> scheduled at kernel start)
        for _ in range(NWARM):
            nc.tensor.matmul(ps_w[:], c_ident[:], c_ident[:], start=True, stop=True)

        # collective warm-up: dummy AllReduce at t~0 (input = const tile, no deps)
        wc_in = dram.tile([1, 128], F32, tag="wc_in")
        wc_out = dram.tile([1, 128], F32, tag="wc_out")
        nc.scalar.dma_start(wc_in[:], c_eye[0:1, :])
        nc.gpsimd.collective_compute(
            "AllReduce", mybir.AluOpType.max,
            replica_groups=[list(range(ncores))],
            ins=[wc_in.opt()], outs=[wc_out.opt()],
        )

        # ================= phase 1 =================
        with ExitStack() as ph1:
            pin = ph1.enter_context(tc.tile_pool(name="pin", bufs=4))
            kpool = ph1.enter_context(tc.tile_pool(name="keys", bufs=4))
            ps_kh = ph1.enter_context(tc.tile_pool(name="ps_kh", bufs=3, space="PSUM"))

            for g in range(ngroups):
                c0 = g * G
                t1g = pin.tile([H, G * W], F8, tag="t1g")
                nc.scalar.dma_start(t1g[:].rearrange("p (c w) -> p c w", c=G),
                                    t1b[:, c0:c0 + G, :])
                t2qg = resq.tile([H, G * W], F16, tag="t2qg")
                nc.sync.dma_start(t2qg[:].rearrange("p (c w) -> p c w", c=G),
                                  t2h[:, c0:c0 + G, :])
                t2q_tiles.append(t2qg)
                t2tg = resqT.tile([W, G * H], BF16, tag="t2tg")
                nc.sync.dma_start(t2tg[:].rearrange("p (c h) -> p c h", c=G),
                                  t2t[:, c0:c0 + G, :])
                t2t_tiles.append(t2tg)

                # kW[c][h,k] (fp16)
                kWr = kpool.tile([H, G * KW], F32, tag="kWr")
                nc.vector.tensor_reduce(
                    kWr[:].rearrange("p (c k) -> p c k", c=G),
                    t1g[:].rearrange("p (c k j) -> p c k j", c=G, j=POOL),
                    axis=mybir.AxisListType.X, op=mybir.AluOpType.add)
                kW = kpool.tile([H, G * KW], F16, tag="kW")
                nc.vector.tensor_scalar_mul(kW[:], kWr[:], 1.0 / POOL)

                # kH[c][w,k] (bf16) = t1b[c].T @ pool16
                ps_kh_t = ps_kh.tile([W, G * KH], F32, tag="ps_kh")
                for i in range(G):
                    nc.tensor.matmul(ps_kh_t[:, i * KH:(i + 1) * KH],
                                     t1g[:, i * W:(i + 1) * W], c_pool16[:],
                                     start=True, stop=True)
                kH = kpool.tile([W, G * KH], BF16, tag="kH")
                nc.vector.tensor_copy(kH[:], ps_kh_t[:])

                first = (g == 0)
                last = (g == ngroups - 1)
                for i in range(G):
                    # eW[w,k] += t2h[c].T @ kW[c]
                    nc.tensor.matmul(ps_eW[:], t2qg[:, i * W:(i + 1) * W],
                                     kW[:, i * KW:(i + 1) * KW],
                                     start=(first and i == 0), stop=(last and i == G - 1))
                    # eH[h,k] += t2t[c].T @ kH[c]
                    nc.tensor.matmul(ps_eH[:], t2tg[:, i * H:(i + 1) * H],
                                     kH[:, i * KH:(i + 1) * KH],
                                     start=(first and i == 0), stop=(last and i == G - 1))

        # ================= boundary =================
        e_sb = spool.tile([H, 2 * KH], F32, tag="e_sb")
        nc.vector.tensor_copy(e_sb[:, 0:KH], ps_eH[:])
        nc.vector.tensor_copy(e_sb[:, KH:2 * KH], ps_eW[:])

        # local (max, -min) on every partition, then all-partition max
        pack = spool.tile([H, 2], F32, tag="pack")
        nc.vector.tensor_reduce(pack[:, 0:1], e_sb[:], axis=mybir.AxisListType.X,
                                op=mybir.AluOpType.max)
        rmin = spool.tile([H, 1], F32, tag="rmin")
        nc.vector.tensor_reduce(rmin[:], e_sb[:], axis=mybir.AxisListType.X,
                                op=mybir.AluOpType.min)
        nc.vector.tensor_scalar_mul(pack[:, 1:2], rmin[:], -1.0)
        packr = spool.tile([H, 2], F32, tag="packr")
        nc.gpsimd.partition_all_reduce(packr[:], pack[:], channels=H,
                                       reduce_op=bass_isa.ReduceOp.max)

        cbuf = spool.tile([1, 128], F32, tag="cbuf")
        nc.vector.memset(cbuf[:], -3.0e38)
        nc.gpsimd.tensor_copy(cbuf[:, 0:2], packr[0:1, :])
        cc_in = dram.tile([1, 128], F32, tag="cc_in")
        cc_out = dram.tile([1, 128], F32, tag="cc_out")
        nc.scalar.dma_start(cc_in[:], cbuf[:])
        nc.gpsimd.collective_compute(
            "AllReduce", mybir.AluOpType.max,
            replica_groups=[list(range(ncores))],
            ins=[cc_in.opt()], outs=[cc_out.opt()],
        )
        g2 = spool.tile([1, 2], F32, tag="g2")
        nc.gpsimd.dma_start(g2[:], cc_out[:, 0:2])

        # PE warm-up burst #2: depends on g2 -> spins PE during softmax chain
        wtile = spool.tile([H, W], BF16, tag="wtile")
        nc.vector.memset(wtile[:], 0.0)
        g2b = spool.tile([1, 1], BF16, tag="g2b")
        nc.vector.tensor_copy(g2b[:], g2[:, 0:1])
        nc.gpsimd.partition_broadcast(wtile[:, 0:1], g2b[:])
        for _ in range(NWARM):
            nc.tensor.matmul(ps_w[:], c_ident[:], wtile[:], start=True, stop=True)

        # broadcast (gmax, -gmin) to all partitions; vectorized softmax prep
        g128 = spool.tile([H, 2], F32, tag="g128")
        nc.gpsimd.partition_broadcast(g128[:], g2[:])
        rng_t = spool.tile([H, 1], F32, tag="rng")
        nc.vector.tensor_tensor(rng_t[:], g128[:, 0:1], g128[:, 1:2],
                                mybir.AluOpType.add)
        inv_t = spool.tile([H, 1], F32, tag="inv")
        nc.vector.reciprocal(inv_t[:], rng_t[:])
        bias_t = spool.tile([H, 1], F32, tag="bias")
        nc.vector.tensor_tensor(bias_t[:], g128[:, 1:2], inv_t[:],
                                mybir.AluOpType.mult)

        s_sb = spool.tile([H, 2 * KH], F32, tag="s_sb")
        ssum = spool.tile([H, 1], F32, tag="ssum")
        nc.scalar.activation(s_sb[:], e_sb[:], mybir.ActivationFunctionType.Exp,
                             bias=bias_t[:], scale=inv_t[:], accum_out=ssum[:])
        stot = spool.tile([H, 1], F32, tag="stot")
        nc.gpsimd.partition_all_reduce(stot[:], ssum[:], channels=H,
                                       reduce_op=bass_isa.ReduceOp.add)
        rn = spool.tile([H, 1], F32, tag="rn")
        nc.vector.reciprocal(rn[:], stot[:])
        s16 = spool.tile([H, 2 * KH], BF16, tag="s16")
        nc.vector.tensor_scalar_mul(s16[:], s_sb[:], rn[:])

        # att transposes + A-mat builds
        apool = top.enter_context(tc.tile_pool(name="amats", bufs=1))
        with tc.tile_pool(name="ps_a", bufs=1, space="PSUM") as ps_a:
            ps_tH = ps_a.tile([KH, H], BF16, tag="ps_tH")
            nc.tensor.transpose(ps_tH[:], s16[:, 0:KH], c_ident[:])
            att_kh = spool.tile([KH, H], BF16, tag="att_kh")
            nc.scalar.copy(att_kh[:], ps_tH[:])
            ps_tW = ps_a.tile([KH, W], BF16, tag="ps_tW")
            nc.tensor.transpose(ps_tW[:], s16[:, KH:2 * KH], c_ident[:])
            att_kw = spool.tile([KH, W], BF16, tag="att_kw")
            nc.scalar.copy(att_kw[:], ps_tW[:])

            ps_ah = ps_a.tile([H, H], F32, tag="ps_ah")
            nc.tensor.matmul(ps_ah[:], c_expmat[:], att_kh[:], start=True, stop=True)
            A_HT = apool.tile([H, H], F16, tag="A_HT")
            nc.vector.scalar_tensor_tensor(A_HT[:], ps_ah[:], 1.0, c_eye[:],
                                           op0=mybir.AluOpType.mult,
                                           op1=mybir.AluOpType.add)
            ps_mw = ps_a.tile([W, W], F32, tag="ps_mw")
            nc.tensor.matmul(ps_mw[:], c_expmat[:], att_kw[:], start=True, stop=True)
            M_W = apool.tile([W, W], BF16, tag="M_W")
            nc.scalar.copy(M_W[:], ps_mw[:])

        psb.close()

        # ================= phase 2 =================
        with ExitStack() as ph2:
            ps_out = ph2.enter_context(tc.tile_pool(name="ps_out", bufs=4, space="PSUM"))
            opool = ph2.enter_context(tc.tile_pool(name="outp", bufs=4))
            for g in range(ngroups):
                c0 = g * G
                t2qg, t2tg = t2q_tiles[g], t2t_tiles[g]
                ps_o = ps_out.tile([H, G * W], F32, tag="ps_o")
                nc.tensor.matmul(ps_o[:, 0:512], A_HT[:], t2qg[:, 0:512],
                                 start=True, stop=False)
                nc.tensor.matmul(ps_o[:, 512:1024], A_HT[:], t2qg[:, 512:1024],
                                 start=True, stop=False)
                for i in range(G):
                    nc.tensor.matmul(ps_o[:, i * W:(i + 1) * W],
                                     t2tg[:, i * H:(i + 1) * H], M_W[:],
                                     start=False, stop=(i % 4 == 3))
                ob = opool.tile([H, G * W], F16, tag="ob")
                half = G * W // 2
                nc.scalar.copy(ob[:, 0:half], ps_o[:, 0:half])
                nc.sync.dma_start(out[:, c0:c0 + G // 2, :],
                                  ob[:, 0:half].rearrange("p (c w) -> p c w", c=G // 2))
                nc.vector.tensor_copy(ob[:, half:], ps_o[:, half:])
                nc.sync.dma_start(out[:, c0 + G // 2:c0 + G, :],
                                  ob[:, half:].rearrange("p (c w) -> p c w", c=G // 2))

    nc.compile()
    return nc


_NC_CACHE = {}


def _get_nc():
    key = (C, NCORES)
    if key not in _NC_CACHE:
        _NC_CACHE[key] = build(C, NCORES)
    return _NC_CACHE[key]


def _stage(tensor1, tensor2):
    """Host-side precision/layout staging for all cores."""
    t1b = np.ascontiguousarray(
        tensor1.astype(F8_NP).transpose(0, 2, 1, 3))            # (B,H,C,W) fp8
    t2h = np.ascontiguousarray(
        tensor2.astype(np.float16).transpose(0, 2, 1, 3))       # (B,H,C,W) fp16
    t2t = np.ascontiguousarray(
        tensor2.astype(BF_NP).transpose(0, 3, 1, 2))            # (B,W,C,H) bf16
    return t1b, t2h, t2t


def kernel(tensor1: np.ndarray, tensor2: np.ndarray) -> np.ndarray:
    from concourse.bass_utils import run_bass_kernel_spmd
    assert tensor1.shape == (B, C, H, W) and tensor2.shape == (B, C, H, W)
    nc = _get_nc()
    consts = host_constants()
    t1b, t2h, t2t = _stage(np.asarray(tensor1, np.float32),
                           np.asarray(tensor2, np.float32))
    in_maps = [
        {"t1b": t1b[b], "t2h": t2h[b], "t2t": t2t[b], **consts}
        for b in range(B)
    ]
    res = run_bass_kernel_spmd(nc, in_maps, core_ids=list(range(NCORES)))
    out_hcw = np.stack([res.results[b]["out"] for b in range(B)])  # (B,H,C,W) f16
    return np.ascontiguousarray(
        out_hcw.transpose(0, 2, 1, 3).astype(np.float32))


# revision 15
# speedup vs baseline: 1.0203x; 1.0203x over previous
"""Criss-cross attention (nn_CC_attention) Trainium2 kernel.

Sharding: pure data parallel over batch B=8 across 8 NeuronCores; the only
cross-core coupling is the global min/max of energy, exchanged via a tiny
AllReduce(max) of (max, -min).

Host-side staging (layout/precision only; all model compute is on-device):
  t1b = bf16(tensor1) as (H, C, W)   -- keys source
  t2h = fp16(tensor2) as (H, C, W)   -- carries the exact +tensor2 path
  t2t = bf16(tensor2) as (W, C, H)   -- pre-transposed copy for the W branch
  out is produced as fp16 (H, C, W), host transposes back to (C, H, W) fp32.

Per-core device algorithm:
  phase 1 (stream 32 groups of 8 channels):
    kW[c][h,k] = avg-pool_w(t1)  (DVE reduce + scale->fp16)
    kH[c][w,k] = t1b[c].T @ P    (PE, t1 as stationary; P = pooling matrix /8)
    eW[w,k] += t2h[c].T @ kW[c]  (PE, t2h stationary, K=h)
    eH[h,k] += t2t[c].T @ kH[c]  (PE, t2t stationary, K=w)
  boundary:
    local (max,-min) -> AllReduce(max) -> global range; exp on ACT; sums;
    att -> A_HT[h',h] = 0.0625*att_H[h,h'//8] + I (fp16)
           M_W[w',w]  = 0.0625*att_W[w,w'//8]     (bf16)
    (0.5 gamma is folded into the 0.0625; the full +tensor2 rides A_HT's I)
  phase 2 (per group):
    psum[h,(c,w)] = A_HT.T @ t2h[g]  (N=512 x2)
                  += t2t[c].T @ M_W  (per c)
    = 0.5*out_H + 0.5*out_W + tensor2 ;  ACT copy -> fp16 -> DMA out.
PE warm-up bursts (dummy matmuls) run at kernel start and during the
collective so the HAM clock gate is open (2.4 GHz) for both phases.
"""

import numpy as np
from contextlib import ExitStack

import ml_dtypes
import concourse.bass as bass
import concourse.tile as tile
from concourse import bacc, bass_isa, mybir

B, C, H, W, POOL = 8, 256, 128, 128, 8
KH, KW = H // POOL, W // POOL  # 16, 16
NCORES = 8
G = 8  # channels per group tile
NWARM = 56  # dummy matmuls per PE warm-up burst

F32 = mybir.dt.float32
F16 = mybir.dt.float16
BF16 = mybir.dt.bfloat16
F8 = mybir.dt.float8e4
BF_NP = ml_dtypes.bfloat16
F8_NP = ml_dtypes.float8_e4m3


def host_constants():
    pool_m = np.zeros((H, KH), np.float32)
    for k in range(KH):
        pool_m[k * POOL:(k + 1) * POOL, k] = 1.0 / POOL
    expmat = np.zeros((KH, H), np.float32)
    for k in range(KH):
        expmat[k, k * POOL:(k + 1) * POOL] = 0.5 / POOL  # 0.0625
    return {
        "pool16": pool_m.astype(F8_NP),
        "ident16": np.eye(H, dtype=np.float32).astype(BF_NP),
        "expmat": expmat.astype(BF_NP),
        "eyefull": np.eye(H, dtype=np.float32),
    }


def build(c_total=C, ncores=NCORES):
    assert c_total % G == 0
    ngroups = c_total // G
    nc = bacc.Bacc(trn_type="TRN2", target_bir_lowering=False, debug=False,
                   num_devices=ncores)

    t1b = nc.dram_tensor("t1b", [H, c_total, W], F8, kind="ExternalInput").ap()
    t2h = nc.dram_tensor("t2h", [H, c_total, W], F16, kind="ExternalInput").ap()
    t2t = nc.dram_tensor("t2t", [W, c_total, H], BF16, kind="ExternalInput").ap()
    pool16 = nc.dram_tensor("pool16", [H, KH], F8, kind="ExternalInput").ap()
    ident16 = nc.dram_tensor("ident16", [H, W], BF16, kind="ExternalInput").ap()
    expmat = nc.dram_tensor("expmat", [KH, H], BF16, kind="ExternalInput").ap()
    eyefull = nc.dram_tensor("eyefull", [H, W], F32, kind="ExternalInput").ap()
    out = nc.dram_tensor("out", [H, c_total, W], F16, kind="ExternalOutput").ap()

    with tile.TileContext(nc) as tc, ExitStack() as top:
        # ---- constants ----
        cpool = top.enter_context(tc.tile_pool(name="consts", bufs=1))
        c_pool16 = cpool.tile([H, KH], F8, tag="pool16")
        nc.sync.dma_start(c_pool16[:], pool16[:])
        c_ident = cpool.tile([H, W], BF16, tag="ident16")
        nc.sync.dma_start(c_ident[:], ident16[:])
        c_expmat = cpool.tile([KH, H], BF16, tag="expmat")
        nc.sync.dma_start(c_expmat[:], expmat[:])
        c_eye = cpool.tile([H, W], F32, tag="eyefull")
        nc.sync.dma_start(c_eye[:], eyefull[:])

        resq = top.enter_context(tc.tile_pool(name="resq", bufs=ngroups))
        resqT = top.enter_context(tc.tile_pool(name="resqT", bufs=ngroups))
        t2q_tiles, t2t_tiles = [], []

        psb = ExitStack()  # psum pools released before phase 2
        ps_e = psb.enter_context(tc.tile_pool(name="ps_e", bufs=1, space="PSUM"))
        ps_eW = ps_e.tile([W, KW], F32, tag="eW")
        ps_eH = ps_e.tile([H, KH], F32, tag="eH")
        ps_warm = psb.enter_context(tc.tile_pool(name="ps_warm", bufs=1, space="PSUM"))
        ps_w = ps_warm.tile([H, W], F32, tag="warm")

        spool = top.enter_context(tc.tile_pool(name="soft", bufs=1))
        dram = top.enter_context(tc.tile_pool(name="dram", bufs=1, space="DRAM"))

        # PE warm-up burst #1 (no data deps -> scheduled at kernel start)
        for _ in range(NWARM):
            nc.tensor.matmul(ps_w[:], c_ident[:], c_ident[:], start=True, stop=True)

        # collective warm-up: dummy AllReduce at t~0 (input = const tile, no deps)
        wc_in = dram.tile([1, 128], F32, tag="wc_in")
        wc_out = dram.tile([1, 128], F32, tag="wc_out")
        nc.sync.dma_start(wc_in[:], c_eye[0:1, :])
        nc.gpsimd.collective_compute(
            "AllReduce", mybir.AluOpType.max,
            replica_groups=[list(range(ncores))],
            ins=[wc_in.opt()], outs=[wc_out.opt()],
        )

        # ================= phase 1 =================
        with ExitStack() as ph1:
            pin = ph1.enter_context(tc.tile_pool(name="pin", bufs=4))
            kpool = ph1.enter_context(tc.tile_pool(name="keys", bufs=4))
            ps_kh = ph1.enter_context(tc.tile_pool(name="ps_kh", bufs=3, space="PSUM"))

            for g in range(ngroups):
                c0 = g * G
                t1g = pin.tile([H, G * W], F8, tag="t1g")
                nc.scalar.dma_start(t1g[:].rearrange("p (c w) -> p c w", c=G),
                                    t1b[:, c0:c0 + G, :])
                t2qg = resq.tile([H, G * W], F16, tag="t2qg")
                nc.sync.dma_start(t2qg[:].rearrange("p (c w) -> p c w", c=G),
                                  t2h[:, c0:c0 + G, :])
                t2q_tiles.append(t2qg)
                t2tg = resqT.tile([W, G * H], BF16, tag="t2tg")
                nc.sync.dma_start(t2tg[:].rearrange("p (c h) -> p c h", c=G),
                                  t2t[:, c0:c0 + G, :])
                t2t_tiles.append(t2tg)

                # kW[c][h,k] (fp16)
                kWr = kpool.tile([H, G * KW], F32, tag="kWr")
                nc.vector.tensor_reduce(
                    kWr[:].rearrange("p (c k) -> p c k", c=G),
                    t1g[:].rearrange("p (c k j) -> p c k j", c=G, j=POOL),
                    axis=mybir.AxisListType.X, op=mybir.AluOpType.add)
                kW = kpool.tile([H, G * KW], F16, tag="kW")
                nc.vector.tensor_scalar_mul(kW[:], kWr[:], 1.0 / POOL)

                # kH[c][w,k] (bf16) = t1b[c].T @ pool16
                ps_kh_t = ps_kh.tile([W, G * KH], F32, tag="ps_kh")
                for i in range(G):
                    nc.tensor.matmul(ps_kh_t[:, i * KH:(i + 1) * KH],
                                     t1g[:, i * W:(i + 1) * W], c_pool16[:],
                                     start=True, stop=True)
                kH = kpool.tile([W, G * KH], BF16, tag="kH")
                nc.vector.tensor_copy(kH[:], ps_kh_t[:])

                first = (g == 0)
                last = (g == ngroups - 1)
                for i in range(G):
                    # eW[w,k] += t2h[c].T @ kW[c]
                    nc.tensor.matmul(ps_eW[:], t2qg[:, i * W:(i + 1) * W],
                                     kW[:, i * KW:(i + 1) * KW],
                                     start=(first and i == 0), stop=(last and i == G - 1))
                    # eH[h,k] += t2t[c].T @ kH[c]
                    nc.tensor.matmul(ps_eH[:], t2tg[:, i * H:(i + 1) * H],
                                     kH[:, i * KH:(i + 1) * KH],
                                     start=(first and i == 0), stop=(last and i == G - 1))

        # ================= boundary =================
        e_sb = spool.tile([H, 2 * KH], F32, tag="e_sb")
        nc.vector.tensor_copy(e_sb[:, 0:KH], ps_eH[:])
        nc.vector.tensor_copy(e_sb[:, KH:2 * KH], ps_eW[:])

        # local (max, -min) on every partition, then all-partition max
        pack = spool.tile([H, 2], F32, tag="pack")
        nc.vector.tensor_reduce(pack[:, 0:1], e_sb[:], axis=mybir.AxisListType.X,
                                op=mybir.AluOpType.max)
        rmin = spool.tile([H, 1], F32, tag="rmin")
        nc.vector.tensor_reduce(rmin[:], e_sb[:], axis=mybir.AxisListType.X,
                                op=mybir.AluOpType.min)
        nc.vector.tensor_scalar_mul(pack[:, 1:2], rmin[:], -1.0)
        packr = spool.tile([H, 2], F32, tag="packr")
        nc.gpsimd.partition_all_reduce(packr[:], pack[:], channels=H,
                                       reduce_op=bass_isa.ReduceOp.max)

        cbuf = spool.tile([1, 128], F32, tag="cbuf")
        nc.vector.memset(cbuf[:], -3.0e38)
        nc.gpsimd.tensor_copy(cbuf[:, 0:2], packr[0:1, :])
        cc_in = dram.tile([1, 128], F32, tag="cc_in")
        cc_out = dram.tile([1, 128], F32, tag="cc_out")
        nc.scalar.dma_start(cc_in[:], cbuf[:])
        nc.gpsimd.collective_compute(
            "AllReduce", mybir.AluOpType.max,
            replica_groups=[list(range(ncores))],
            ins=[cc_in.opt()], outs=[cc_out.opt()],
        )
        g2 = spool.tile([1, 2], F32, tag="g2")
        nc.scalar.dma_start(g2[:], cc_out[:, 0:2])

        # PE warm-up burst #2: depends on g2 -> spins PE during softmax chain
        wtile = spool.tile([H, W], BF16, tag="wtile")
        nc.vector.memset(wtile[:], 0.0)
        g2b = spool.tile([1, 1], BF16, tag="g2b")
        nc.vector.tensor_copy(g2b[:], g2[:, 0:1])
        nc.gpsimd.partition_broadcast(wtile[:, 0:1], g2b[:])
        for _ in range(NWARM):
            nc.tensor.matmul(ps_w[:], c_ident[:], wtile[:], start=True, stop=True)

        # broadcast (gmax, -gmin) to all partitions; vectorized softmax prep
        g128 = spool.tile([H, 2], F32, tag="g128")
        nc.gpsimd.partition_broadcast(g128[:], g2[:])
        rng_t = spool.tile([H, 1], F32, tag="rng")
        nc.vector.tensor_tensor(rng_t[:], g128[:, 0:1], g128[:, 1:2],
                                mybir.AluOpType.add)
        inv_t = spool.tile([H, 1], F32, tag="inv")
        nc.vector.reciprocal(inv_t[:], rng_t[:])
        bias_t = spool.tile([H, 1], F32, tag="bias")
        nc.vector.tensor_tensor(bias_t[:], g128[:, 1:2], inv_t[:],
                                mybir.AluOpType.mult)

        s_sb = spool.tile([H, 2 * KH], F32, tag="s_sb")
        ssum = spool.tile([H, 1], F32, tag="ssum")
        nc.scalar.activation(s_sb[:], e_sb[:], mybir.ActivationFunctionType.Exp,
                             bias=bias_t[:], scale=inv_t[:], accum_out=ssum[:])
        stot = spool.tile([H, 1], F32, tag="stot")
        nc.gpsimd.partition_all_reduce(stot[:], ssum[:], channels=H,
                                       reduce_op=bass_isa.ReduceOp.add)
        rn = spool.tile([H, 1], F32, tag="rn")
        nc.vector.reciprocal(rn[:], stot[:])
        s16 = spool.tile([H, 2 * KH], BF16, tag="s16")
        nc.vector.tensor_scalar_mul(s16[:], s_sb[:], rn[:])

        # att transposes + A-mat builds
        apool = top.enter_context(tc.tile_pool(name="amats", bufs=1))
        with tc.tile_pool(name="ps_a", bufs=1, space="PSUM") as ps_a:
            ps_tH = ps_a.tile([KH, H], BF16, tag="ps_tH")
            nc.tensor.transpose(ps_tH[:], s16[:, 0:KH], c_ident[:])
            att_kh = spool.tile([KH, H], BF16, tag="att_kh")
            nc.scalar.copy(att_kh[:], ps_tH[:])
            ps_tW = ps_a.tile([KH, W], BF16, tag="ps_tW")
            nc.tensor.transpose(ps_tW[:], s16[:, KH:2 * KH], c_ident[:])
            att_kw = spool.tile([KH, W], BF16, tag="att_kw")
            nc.scalar.copy(att_kw[:], ps_tW[:])

            ps_ah = ps_a.tile([H, H], F32, tag="ps_ah")
            nc.tensor.matmul(ps_ah[:], c_expmat[:], att_kh[:], start=True, stop=True)
            A_HT = apool.tile([H, H], F16, tag="A_HT")
            nc.vector.scalar_tensor_tensor(A_HT[:], ps_ah[:], 1.0, c_eye[:],
                                           op0=mybir.AluOpType.mult,
                                           op1=mybir.AluOpType.add)
            ps_mw = ps_a.tile([W, W], F32, tag="ps_mw")
            nc.tensor.matmul(ps_mw[:], c_expmat[:], att_kw[:], start=True, stop=True)
            M_W = apool.tile([W, W], BF16, tag="M_W")
            nc.scalar.copy(M_W[:], ps_mw[:])

        psb.close()

        # ================= phase 2 =================
        with ExitStack() as ph2:
            ps_out = ph2.enter_context(tc.tile_pool(name="ps_out", bufs=4, space="PSUM"))
            opool = ph2.enter_context(tc.tile_pool(name="outp", bufs=4))
            for g in range(ngroups):
                c0 = g * G
                t2qg, t2tg = t2q_tiles[g], t2t_tiles[g]
                ps_o = ps_out.tile([H, G * W], F32, tag="ps_o")
                nc.tensor.matmul(ps_o[:, 0:512], A_HT[:], t2qg[:, 0:512],
                                 start=True, stop=False)
                nc.tensor.matmul(ps_o[:, 512:1024], A_HT[:], t2qg[:, 512:1024],
                                 start=True, stop=False)
                for i in range(G):
                    nc.tensor.matmul(ps_o[:, i * W:(i + 1) * W],
                                     t2tg[:, i * H:(i + 1) * H], M_W[:],
                                     start=False, stop=(i % 4 == 3))
                ob = opool.tile([H, G * W], F16, tag="ob")
                half = G * W // 2
                nc.scalar.copy(ob[:, 0:half], ps_o[:, 0:half])
                nc.sync.dma_start(out[:, c0:c0 + G // 2, :],
                                  ob[:, 0:half].rearrange("p (c w) -> p c w", c=G // 2))
                nc.vector.tensor_copy(ob[:, half:], ps_o[:, half:])
                nc.sync.dma_start(out[:, c0 + G // 2:c0 + G, :],
                                  ob[:, half:].rearrange("p (c w) -> p c w", c=G // 2))

    nc.compile()
    return nc


_NC_CACHE = {}


def _get_nc():
    key = (C, NCORES)
    if key not in _NC_CACHE:
        _NC_CACHE[key] = build(C, NCORES)
    return _NC_CACHE[key]


def _stage(tensor1, tensor2):
    """Host-side precision/layout staging for all cores."""
    t1b = np.ascontiguousarray(
        tensor1.astype(F8_NP).transpose(0, 2, 1, 3))            # (B,H,C,W) fp8
    t2h = np.ascontiguousarray(
        tensor2.astype(np.float16).transpose(0, 2, 1, 3))       # (B,H,C,W) fp16
    t2t = np.ascontiguousarray(
        tensor2.astype(BF_NP).transpose(0, 3, 1, 2))            # (B,W,C,H) bf16
    return t1b, t2h, t2t


def kernel(tensor1: np.ndarray, tensor2: np.ndarray) -> np.ndarray:
    from concourse.bass_utils import run_bass_kernel_spmd
    assert tensor1.shape == (B, C, H, W) and tensor2.shape == (B, C, H, W)
    nc = _get_nc()
    consts = host_constants()
    t1b, t2h, t2t = _stage(np.asarray(tensor1, np.float32),
                           np.asarray(tensor2, np.float32))
    in_maps = [
        {"t1b": t1b[b], "t2h": t2h[b], "t2t": t2t[b], **consts}
        for b in range(B)
    ]
    res = run_bass_kernel_spmd(nc, in_maps, core_ids=list(range(NCORES)))
    out_hcw = np.stack([res.results[b]["out"] for b in range(B)])  # (B,H,C,W) f16
    return np.ascontiguousarray(
        out_hcw.transpose(0, 2, 1, 3).astype(np.float32))


# revision 16
# speedup vs baseline: 1.0823x; 1.0608x over previous
"""Criss-cross attention (nn_CC_attention) Trainium2 kernel.

Sharding: pure data parallel over batch B=8 across 8 NeuronCores; the only
cross-core coupling is the global min/max of energy, exchanged via a tiny
AllReduce(max) of (max, -min).

Host-side staging (layout/precision only; all model compute is on-device):
  t1b = bf16(tensor1) as (H, C, W)   -- keys source
  t2h = fp16(tensor2) as (H, C, W)   -- carries the exact +tensor2 path
  t2t = bf16(tensor2) as (W, C, H)   -- pre-transposed copy for the W branch
  out is produced as fp16 (H, C, W), host transposes back to (C, H, W) fp32.

Per-core device algorithm:
  phase 1 (stream 32 groups of 8 channels):
    kW[c][h,k] = avg-pool_w(t1)  (DVE reduce + scale->fp16)
    kH[c][w,k] = t1b[c].T @ P    (PE, t1 as stationary; P = pooling matrix /8)
    eW[w,k] += t2h[c].T @ kW[c]  (PE, t2h stationary, K=h)
    eH[h,k] += t2t[c].T @ kH[c]  (PE, t2t stationary, K=w)
  boundary:
    local (max,-min) -> AllReduce(max) -> global range; exp on ACT; sums;
    att -> A_HT[h',h] = 0.0625*att_H[h,h'//8] + I (fp16)
           M_W[w',w]  = 0.0625*att_W[w,w'//8]     (bf16)
    (0.5 gamma is folded into the 0.0625; the full +tensor2 rides A_HT's I)
  phase 2 (per group):
    psum[h,(c,w)] = A_HT.T @ t2h[g]  (N=512 x2)
                  += t2t[c].T @ M_W  (per c)
    = 0.5*out_H + 0.5*out_W + tensor2 ;  ACT copy -> fp16 -> DMA out.
PE warm-up bursts (dummy matmuls) run at kernel start and during the
collective so the HAM clock gate is open (2.4 GHz) for both phases.
"""

import numpy as np
from contextlib import ExitStack

import ml_dtypes
import concourse.bass as bass
import concourse.tile as tile
from concourse import bacc, bass_isa, mybir

B, C, H, W, POOL = 8, 256, 128, 128, 8
KH, KW = H // POOL, W // POOL  # 16, 16
NCORES = 8
G = 16  # channels per group tile
NWARM = 56  # dummy matmuls per PE warm-up burst

F32 = mybir.dt.float32
F16 = mybir.dt.float16
BF16 = mybir.dt.bfloat16
F8 = mybir.dt.float8e4
BF_NP = ml_dtypes.bfloat16
F8_NP = ml_dtypes.float8_e4m3


def host_constants():
    pool_m = np.zeros((H, KH), np.float32)
    for k in range(KH):
        pool_m[k * POOL:(k + 1) * POOL, k] = 1.0 / POOL
    expmat = np.zeros((KH, H), np.float32)
    for k in range(KH):
        expmat[k, k * POOL:(k + 1) * POOL] = 0.5 / POOL  # 0.0625
    return {
        "pool16": pool_m.astype(F8_NP),
        "ident16": np.eye(H, dtype=np.float32).astype(BF_NP),
        "expmat": expmat.astype(BF_NP),
        "eyefull": np.eye(H, dtype=np.float32),
    }


def build(c_total=C, ncores=NCORES):
    assert c_total % G == 0
    ngroups = c_total // G
    nc = bacc.Bacc(trn_type="TRN2", target_bir_lowering=False, debug=False,
                   num_devices=ncores)

    t1b = nc.dram_tensor("t1b", [H, c_total, W], F8, kind="ExternalInput").ap()
    t2h = nc.dram_tensor("t2h", [H, c_total, W], F16, kind="ExternalInput").ap()
    t2t = nc.dram_tensor("t2t", [W, c_total, H], BF16, kind="ExternalInput").ap()
    pool16 = nc.dram_tensor("pool16", [H, KH], F8, kind="ExternalInput").ap()
    ident16 = nc.dram_tensor("ident16", [H, W], BF16, kind="ExternalInput").ap()
    expmat = nc.dram_tensor("expmat", [KH, H], BF16, kind="ExternalInput").ap()
    eyefull = nc.dram_tensor("eyefull", [H, W], F32, kind="ExternalInput").ap()
    out = nc.dram_tensor("out", [H, c_total, W], F16, kind="ExternalOutput").ap()

    with tile.TileContext(nc) as tc, ExitStack() as top:
        # ---- constants ----
        cpool = top.enter_context(tc.tile_pool(name="consts", bufs=1))
        c_pool16 = cpool.tile([H, KH], F8, tag="pool16")
        nc.sync.dma_start(c_pool16[:], pool16[:])
        c_ident = cpool.tile([H, W], BF16, tag="ident16")
        nc.sync.dma_start(c_ident[:], ident16[:])
        c_expmat = cpool.tile([KH, H], BF16, tag="expmat")
        nc.sync.dma_start(c_expmat[:], expmat[:])
        c_eye = cpool.tile([H, W], F32, tag="eyefull")
        nc.sync.dma_start(c_eye[:], eyefull[:])

        resq = top.enter_context(tc.tile_pool(name="resq", bufs=ngroups))
        resqT = top.enter_context(tc.tile_pool(name="resqT", bufs=ngroups))
        t2q_tiles, t2t_tiles = [], []

        psb = ExitStack()  # psum pools released before phase 2
        ps_e = psb.enter_context(tc.tile_pool(name="ps_e", bufs=1, space="PSUM"))
        ps_eW = ps_e.tile([W, KW], F32, tag="eW")
        ps_eH = ps_e.tile([H, KH], F32, tag="eH")
        ps_warm = psb.enter_context(tc.tile_pool(name="ps_warm", bufs=1, space="PSUM"))
        ps_w = ps_warm.tile([H, W], F32, tag="warm")

        spool = top.enter_context(tc.tile_pool(name="soft", bufs=1))
        dram = top.enter_context(tc.tile_pool(name="dram", bufs=1, space="DRAM"))

        # PE warm-up burst #1 (no data deps -> scheduled at kernel start)
        for _ in range(NWARM):
            nc.tensor.matmul(ps_w[:], c_ident[:], c_ident[:], start=True, stop=True)

        # collective warm-up: dummy AllReduce at t~0 (input = const tile, no deps)
        wc_in = dram.tile([1, 128], F32, tag="wc_in")
        wc_out = dram.tile([1, 128], F32, tag="wc_out")
        nc.sync.dma_start(wc_in[:], c_eye[0:1, :])
        nc.gpsimd.collective_compute(
            "AllReduce", mybir.AluOpType.max,
            replica_groups=[list(range(ncores))],
            ins=[wc_in.opt()], outs=[wc_out.opt()],
        )

        # ================= phase 1 =================
        with ExitStack() as ph1:
            pin = ph1.enter_context(tc.tile_pool(name="pin", bufs=4))
            kpool = ph1.enter_context(tc.tile_pool(name="keys", bufs=4))
            ps_kh = ph1.enter_context(tc.tile_pool(name="ps_kh", bufs=3, space="PSUM"))

            for g in range(ngroups):
                c0 = g * G
                t1g = pin.tile([H, G * W], F8, tag="t1g")
                nc.scalar.dma_start(t1g[:].rearrange("p (c w) -> p c w", c=G),
                                    t1b[:, c0:c0 + G, :])
                t2qg = resq.tile([H, G * W], F16, tag="t2qg")
                nc.sync.dma_start(t2qg[:].rearrange("p (c w) -> p c w", c=G),
                                  t2h[:, c0:c0 + G, :])
                t2q_tiles.append(t2qg)
                t2tg = resqT.tile([W, G * H], BF16, tag="t2tg")
                nc.sync.dma_start(t2tg[:].rearrange("p (c h) -> p c h", c=G),
                                  t2t[:, c0:c0 + G, :])
                t2t_tiles.append(t2tg)

                # kW[c][h,k] (fp16)
                kWr = kpool.tile([H, G * KW], F32, tag="kWr")
                nc.vector.tensor_reduce(
                    kWr[:].rearrange("p (c k) -> p c k", c=G),
                    t1g[:].rearrange("p (c k j) -> p c k j", c=G, j=POOL),
                    axis=mybir.AxisListType.X, op=mybir.AluOpType.add)
                kW = kpool.tile([H, G * KW], F16, tag="kW")
                nc.vector.tensor_scalar_mul(kW[:], kWr[:], 1.0 / POOL)

                # kH[c][w,k] (bf16) = t1b[c].T @ pool16
                ps_kh_t = ps_kh.tile([W, G * KH], F32, tag="ps_kh")
                for i in range(G):
                    nc.tensor.matmul(ps_kh_t[:, i * KH:(i + 1) * KH],
                                     t1g[:, i * W:(i + 1) * W], c_pool16[:],
                                     start=True, stop=True)
                kH = kpool.tile([W, G * KH], BF16, tag="kH")
                nc.vector.tensor_copy(kH[:], ps_kh_t[:])

                first = (g == 0)
                last = (g == ngroups - 1)
                for i in range(G):
                    # eW[w,k] += t2h[c].T @ kW[c]
                    nc.tensor.matmul(ps_eW[:], t2qg[:, i * W:(i + 1) * W],
                                     kW[:, i * KW:(i + 1) * KW],
                                     start=(first and i == 0), stop=(last and i == G - 1))
                    # eH[h,k] += t2t[c].T @ kH[c]
                    nc.tensor.matmul(ps_eH[:], t2tg[:, i * H:(i + 1) * H],
                                     kH[:, i * KH:(i + 1) * KH],
                                     start=(first and i == 0), stop=(last and i == G - 1))

        # ================= boundary =================
        e_sb = spool.tile([H, 2 * KH], F32, tag="e_sb")
        nc.vector.tensor_copy(e_sb[:, 0:KH], ps_eH[:])
        nc.vector.tensor_copy(e_sb[:, KH:2 * KH], ps_eW[:])

        # local (max, -min) on every partition, then all-partition max
        pack = spool.tile([H, 2], F32, tag="pack")
        nc.vector.tensor_reduce(pack[:, 0:1], e_sb[:], axis=mybir.AxisListType.X,
                                op=mybir.AluOpType.max)
        rmin = spool.tile([H, 1], F32, tag="rmin")
        nc.vector.tensor_reduce(rmin[:], e_sb[:], axis=mybir.AxisListType.X,
                                op=mybir.AluOpType.min)
        nc.vector.tensor_scalar_mul(pack[:, 1:2], rmin[:], -1.0)
        packr = spool.tile([H, 2], F32, tag="packr")
        nc.gpsimd.partition_all_reduce(packr[:], pack[:], channels=H,
                                       reduce_op=bass_isa.ReduceOp.max)

        cbuf = spool.tile([1, 128], F32, tag="cbuf")
        nc.vector.memset(cbuf[:], -3.0e38)
        nc.gpsimd.tensor_copy(cbuf[:, 0:2], packr[0:1, :])
        cc_in = dram.tile([1, 128], F32, tag="cc_in")
        cc_out = dram.tile([1, 128], F32, tag="cc_out")
        nc.scalar.dma_start(cc_in[:], cbuf[:])
        nc.gpsimd.collective_compute(
            "AllReduce", mybir.AluOpType.max,
            replica_groups=[list(range(ncores))],
            ins=[cc_in.opt()], outs=[cc_out.opt()],
        )
        g2 = spool.tile([1, 2], F32, tag="g2")
        nc.scalar.dma_start(g2[:], cc_out[:, 0:2])

        # PE warm-up burst #2: depends on g2 -> spins PE during softmax chain
        wtile = spool.tile([H, W], BF16, tag="wtile")
        nc.vector.memset(wtile[:], 0.0)
        g2b = spool.tile([1, 1], BF16, tag="g2b")
        nc.vector.tensor_copy(g2b[:], g2[:, 0:1])
        nc.gpsimd.partition_broadcast(wtile[:, 0:1], g2b[:])
        for _ in range(NWARM):
            nc.tensor.matmul(ps_w[:], c_ident[:], wtile[:], start=True, stop=True)

        # broadcast (gmax, -gmin) to all partitions; vectorized softmax prep
        g128 = spool.tile([H, 2], F32, tag="g128")
        nc.gpsimd.partition_broadcast(g128[:], g2[:])
        rng_t = spool.tile([H, 1], F32, tag="rng")
        nc.vector.tensor_tensor(rng_t[:], g128[:, 0:1], g128[:, 1:2],
                                mybir.AluOpType.add)
        inv_t = spool.tile([H, 1], F32, tag="inv")
        nc.vector.reciprocal(inv_t[:], rng_t[:])
        bias_t = spool.tile([H, 1], F32, tag="bias")
        nc.vector.tensor_tensor(bias_t[:], g128[:, 1:2], inv_t[:],
                                mybir.AluOpType.mult)

        s_sb = spool.tile([H, 2 * KH], F32, tag="s_sb")
        ssum = spool.tile([H, 1], F32, tag="ssum")
        nc.scalar.activation(s_sb[:], e_sb[:], mybir.ActivationFunctionType.Exp,
                             bias=bias_t[:], scale=inv_t[:], accum_out=ssum[:])
        stot = spool.tile([H, 1], F32, tag="stot")
        nc.gpsimd.partition_all_reduce(stot[:], ssum[:], channels=H,
                                       reduce_op=bass_isa.ReduceOp.add)
        rn = spool.tile([H, 1], F32, tag="rn")
        nc.vector.reciprocal(rn[:], stot[:])
        s16 = spool.tile([H, 2 * KH], BF16, tag="s16")
        nc.vector.tensor_scalar_mul(s16[:], s_sb[:], rn[:])

        # att transposes + A-mat builds
        apool = top.enter_context(tc.tile_pool(name="amats", bufs=1))
        with tc.tile_pool(name="ps_a", bufs=1, space="PSUM") as ps_a:
            ps_tH = ps_a.tile([KH, H], BF16, tag="ps_tH")
            nc.tensor.transpose(ps_tH[:], s16[:, 0:KH], c_ident[:])
            att_kh = spool.tile([KH, H], BF16, tag="att_kh")
            nc.scalar.copy(att_kh[:], ps_tH[:])
            ps_tW = ps_a.tile([KH, W], BF16, tag="ps_tW")
            nc.tensor.transpose(ps_tW[:], s16[:, KH:2 * KH], c_ident[:])
            att_kw = spool.tile([KH, W], BF16, tag="att_kw")
            nc.scalar.copy(att_kw[:], ps_tW[:])

            ps_ah = ps_a.tile([H, H], F32, tag="ps_ah")
            nc.tensor.matmul(ps_ah[:], c_expmat[:], att_kh[:], start=True, stop=True)
            A_HT = apool.tile([H, H], F16, tag="A_HT")
            nc.vector.scalar_tensor_tensor(A_HT[:], ps_ah[:], 1.0, c_eye[:],
                                           op0=mybir.AluOpType.mult,
                                           op1=mybir.AluOpType.add)
            ps_mw = ps_a.tile([W, W], F32, tag="ps_mw")
            nc.tensor.matmul(ps_mw[:], c_expmat[:], att_kw[:], start=True, stop=True)
            M_W = apool.tile([W, W], BF16, tag="M_W")
            nc.scalar.copy(M_W[:], ps_mw[:])

        psb.close()

        # ================= phase 2 =================
        with ExitStack() as ph2:
            ps_out = ph2.enter_context(tc.tile_pool(name="ps_out", bufs=2, space="PSUM"))
            opool = ph2.enter_context(tc.tile_pool(name="outp", bufs=4))
            for g in range(ngroups):
                c0 = g * G
                t2qg, t2tg = t2q_tiles[g], t2t_tiles[g]
                ps_o = ps_out.tile([H, G * W], F32, tag="ps_o")
                for j in range(0, G * W, 512):
                    nc.tensor.matmul(ps_o[:, j:j + 512], A_HT[:], t2qg[:, j:j + 512],
                                     start=True, stop=False)
                for i in range(G):
                    nc.tensor.matmul(ps_o[:, i * W:(i + 1) * W],
                                     t2tg[:, i * H:(i + 1) * H], M_W[:],
                                     start=False, stop=(i % 4 == 3))
                ob = opool.tile([H, G * W], F16, tag="ob")
                half = G * W // 2
                nc.scalar.copy(ob[:, 0:half], ps_o[:, 0:half])
                nc.sync.dma_start(out[:, c0:c0 + G // 2, :],
                                  ob[:, 0:half].rearrange("p (c w) -> p c w", c=G // 2))
                nc.vector.tensor_copy(ob[:, half:], ps_o[:, half:])
                nc.sync.dma_start(out[:, c0 + G // 2:c0 + G, :],
                                  ob[:, half:].rearrange("p (c w) -> p c w", c=G // 2))

    nc.compile()
    return nc


_NC_CACHE = {}


def _get_nc():
    key = (C, NCORES)
    if key not in _NC_CACHE:
        _NC_CACHE[key] = build(C, NCORES)
    return _NC_CACHE[key]


def _stage(tensor1, tensor2):
    """Host-side precision/layout staging for all cores."""
    t1b = np.ascontiguousarray(
        tensor1.astype(F8_NP).transpose(0, 2, 1, 3))            # (B,H,C,W) fp8
    t2h = np.ascontiguousarray(
        tensor2.astype(np.float16).transpose(0, 2, 1, 3))       # (B,H,C,W) fp16
    t2t = np.ascontiguousarray(
        tensor2.astype(BF_NP).transpose(0, 3, 1, 2))            # (B,W,C,H) bf16
    return t1b, t2h, t2t


def kernel(tensor1: np.ndarray, tensor2: np.ndarray) -> np.ndarray:
    from concourse.bass_utils import run_bass_kernel_spmd
    assert tensor1.shape == (B, C, H, W) and tensor2.shape == (B, C, H, W)
    nc = _get_nc()
    consts = host_constants()
    t1b, t2h, t2t = _stage(np.asarray(tensor1, np.float32),
                           np.asarray(tensor2, np.float32))
    in_maps = [
        {"t1b": t1b[b], "t2h": t2h[b], "t2t": t2t[b], **consts}
        for b in range(B)
    ]
    res = run_bass_kernel_spmd(nc, in_maps, core_ids=list(range(NCORES)))
    out_hcw = np.stack([res.results[b]["out"] for b in range(B)])  # (B,H,C,W) f16
    return np.ascontiguousarray(
        out_hcw.transpose(0, 2, 1, 3).astype(np.float32))
